# revision 1
# baseline (speedup 1.0000x reference)
"""Trainium2 Bass kernel for nn_AxonalConnections.

Computes, per (batch b, patch n):
    out[t]  = sum_s sp[b,n,s] * W_dyn[b,n,t,s]          (batched matvec, distinct weights)
    out_n   = LayerNorm_T(out) * gamma + beta
    w       = softmax(out_n / TEMP)
    final   = w * (gates[n] * sum_s sp[b,n,s] + biases[n])
    fold -> [B, 256, 256]

Strategy: 8-way shard over (batch b, patch-half); each core owns 128 patches.
The heavy matvec runs on the TensorEngine:
  - host passes W transposed per patch ([n, s, t]) and split into bf16
    hi + lo halves (hi + lo carries ~2^-16 relative error, well inside
    tolerance; bf16 runs the PE at 1 cycle/row vs fp32's 4)
  - lhsT is the whole core's spike matrix SP^T [s, 128 patches]; one matmul
    per (patch-pair, s-half, hi/lo) computes all 128 patch rows of
    SP^T.T @ W_n^T but only row n is meaningful — extra rows cost nothing
    since PE time scales only with the moving free dim
  - accumulation over (s-half, hi/lo) happens in PSUM; a DVE copy extracts
    row n of each patch into the [128 patches, 256] result tile
  - LayerNorm + temperature softmax epilogue on DVE/ACT
Unfold/fold, the W transpose/split, and shard assembly are host-side numpy.
"""

import sys

for _p in ("/opt/trn_rl_repo",):
    if _p not in sys.path:
        sys.path.insert(0, _p)

import numpy as np
import ml_dtypes

import concourse.bass as bass
import concourse.bacc as bacc
import concourse.tile as tile
from concourse import mybir
from concourse import bass_utils

# Problem constants (hardcoded per contract)
B = 4
GRID = 256
PATCH = 16
PH = GRID // PATCH          # 16 patches per side
N = PH * PH                 # 256 patches
S = PATCH * PATCH           # 256 source pixels per patch
T = 256                     # 256 target pixels per patch
TEMP = 0.1
LN_EPS = 1e-5

NCORES = 8
P = 128                     # patches per core (= SBUF partitions)
# W streamed in variable-size patch groups: small leading groups shorten the
# pipeline ramp (matmuls can start after ~0.5MB instead of 3MB)
GROUPS = [4, 4] + [8] * 14 + [4, 4]
LOSH = 12                   # wlo is shipped as fp8e4m3 scaled by 2**LOSH;
                            # the lo-pass lhsT carries 2**-LOSH instead of 1.0

F32 = mybir.dt.float32
BF16 = mybir.dt.bfloat16
NP_BF16 = ml_dtypes.bfloat16

_NC_CACHE = {}


def _build_nc():
    # Bacc (not raw Bass): its compile() runs generate_event_semaphores,
    # which splits multi-sem waits into EventSemaphore instructions — the
    # TRN2 "at most 1 wait per instruction" legalization walrus requires.
    nc = bacc.Bacc("TRN2")
    # W^T hi/lo, pre-packed host-side to the exact SBUF tile layout
    # [g, partition(s%128), (n-in-group, s-half, t)] so each W DMA is a plain
    # [128 x 16KB-contiguous] transfer (512B-run layouts drop DMA to ~275GB/s)
    whi = nc.dram_tensor("whi", [P, P * 2 * T], BF16, kind="ExternalInput")
    wlo = nc.dram_tensor("wlo", [P, P * 2 * T], mybir.dt.float8e4,
                         kind="ExternalInput")
    spt = nc.dram_tensor("spt", [S, P], BF16, kind="ExternalInput")
    sptl = nc.dram_tensor("sptl", [S, P], BF16, kind="ExternalInput")
    sp = nc.dram_tensor("sp", [P, S], F32, kind="ExternalInput")
    # one-hot row masks for the per-pair PSUM row extraction:
    # msk[p, q, i] = 1.0 iff p == 2q + i  (partition offsets must be
    # 32-aligned on trn2, so rows are picked by predicated copies instead)
    msk = nc.dram_tensor("msk", [P, P // 2 + 1, 2], mybir.dt.uint8,
                         kind="ExternalInput")
    # packed per-core params: [gamma/TEMP (256) | beta/TEMP (256) | gate | bias]
    prm = nc.dram_tensor("prm", [P, 2 * T + 2], F32, kind="ExternalInput")
    outd = nc.dram_tensor("out", [P, T], F32, kind="ExternalOutput")

    Alu = mybir.AluOpType
    Act = mybir.ActivationFunctionType
    Ax = mybir.AxisListType

    with tile.TileContext(nc) as tc:
        with (
            tc.tile_pool(name="wpool", bufs=4) as wpool,
            tc.tile_pool(name="pspool", bufs=8, space="PSUM") as pspool,
            tc.tile_pool(name="sing", bufs=1) as sing,
            tc.tile_pool(name="small", bufs=2) as small,
        ):
            # lhsT tiles first (tiny; the first matmul needs them), then the
            # leading W blocks, then the remaining params.
            spt_t = sing.tile([P, 2, P], BF16)
            nc.sync.dma_start(
                out=spt_t,
                in_=spt.rearrange("(sh p) m -> p sh m", p=P))
            sptl_t = sing.tile([P, 2, P], BF16)
            nc.sync.dma_start(
                out=sptl_t,
                in_=sptl.rearrange("(sh p) m -> p sh m", p=P))
            msk_t = sing.tile([P, P // 2 + 1, 2], mybir.dt.uint8)
            nc.scalar.dma_start(out=msk_t, in_=msk[:, :, :])
            sp_t = sing.tile([P, S], F32)
            nc.scalar.dma_start(out=sp_t, in_=sp[:, :])
            prm_t = sing.tile([P, 2 * T + 2], F32)
            nc.scalar.dma_start(out=prm_t, in_=prm[:, :])
            gmt_t = prm_t[:, 0:T]
            bft_t = prm_t[:, T : 2 * T]
            gat_t = prm_t[:, 2 * T : 2 * T + 1]
            bia_t = prm_t[:, 2 * T + 1 : 2 * T + 2]
            eps_t = sing.tile([P, 1], F32)
            nc.vector.memset(eps_t, LN_EPS)

            outm = sing.tile([P, T], F32)     # raw matvec results (n, t)
            outw = sing.tile([P, 2, T], F32)  # wide staging: even rows in
                                              # half 0, odd rows in half 1

            # Absorb the sp/prm DMA completion waits into non-TT DVE ops
            # (plain TensorTensor only survives walrus codegen with <=1 wait).
            spsum = small.tile([P, 1], F32)
            nc.vector.tensor_reduce(out=spsum, in_=sp_t, axis=Ax.X, op=Alu.add)
            touch = small.tile([P, 1], F32)
            nc.vector.tensor_scalar_mul(touch, gmt_t[:, 0:1], 1.0)
            # warm the Exp activation table (otherwise a ~1.3us lazy
            # ACT_TABLE_LOAD lands in the serial epilogue tail)
            warm = small.tile([P, 1], F32)
            nc.scalar.activation(out=warm, in_=eps_t, func=Act.Exp)
            # per-patch scalar chain only needs spsum/gates/biases -> emit
            # early so it never sits in the tail
            scal = small.tile([P, 1], F32)
            nc.vector.tensor_mul(scal, gat_t, spsum)
            scal2 = small.tile([P, 1], F32)
            nc.vector.tensor_add(scal2, scal, bia_t)

            # ---- main pass: stream W^T hi/lo; PE matvec; DVE row extract ----
            n0 = 0
            for g, gp in enumerate(GROUPS):
                cw = 2 * T  # free columns per patch
                whit = wpool.tile([P, gp, 2, T], BF16, tag="whit")
                nc.sync.dma_start(
                    out=whit.rearrange("p a b t -> p (a b t)"),
                    in_=whi[:, n0 * cw : (n0 + gp) * cw])
                wlot = wpool.tile([P, gp, 2, T], mybir.dt.float8e4, tag="wlot")
                nc.scalar.dma_start(
                    out=wlot.rearrange("p a b t -> p (a b t)"),
                    in_=wlo[:, n0 * cw : (n0 + gp) * cw])
                for q in range(gp // 2):
                    ps = pspool.tile([P, 2 * T], F32)
                    for i, (wsrc, lh) in enumerate(((whit, spt_t),
                                                    (wlot, sptl_t))):
                        for sh in range(2):
                            nc.tensor.matmul(
                                ps,
                                lhsT=lh[:, sh, :],
                                rhs=wsrc[:, 2 * q : 2 * q + 2, sh, :],
                                start=(i == 0 and sh == 0),
                                stop=(i == 1 and sh == 1))
                    qg = n0 // 2 + q
                    nc.vector.copy_predicated(
                        out=outw,
                        mask=msk_t[:, qg, :].broadcast_to((P, 2, T)),
                        data=ps.rearrange("p (h t) -> p h t", h=2))
                n0 += gp

            # merge the wide staging into outm (even rows from half 0,
            # odd rows from half 1)
            nc.vector.copy_predicated(
                out=outm,
                mask=msk_t[:, P // 2, 0:1].broadcast_to((P, T)),
                data=outw[:, 0, :])
            nc.vector.copy_predicated(
                out=outm,
                mask=msk_t[:, P // 2, 1:2].broadcast_to((P, T)),
                data=outw[:, 1, :])

            # ---- LayerNorm over t ----
            stats = small.tile([P, 6], F32)
            nc.vector.bn_stats(out=stats, in_=outm)
            mv = small.tile([P, 2], F32)
            nc.vector.bn_aggr(out=mv, in_=stats)
            std = small.tile([P, 1], F32)
            nc.scalar.activation(out=std, in_=mv[:, 1:2], func=Act.Sqrt,
                                 bias=eps_t, scale=1.0)
            rstd = small.tile([P, 1], F32)
            nc.vector.reciprocal(out=rstd, in_=std)
            z1 = small.tile([P, T], F32)
            nc.vector.tensor_scalar(out=z1, in0=outm, scalar1=mv[:, 0:1],
                                    scalar2=rstd, op0=Alu.subtract,
                                    op1=Alu.mult)
            z2 = small.tile([P, T], F32)
            nc.vector.tensor_mul(z2, z1, gmt_t)
            z3 = small.tile([P, T], F32)
            nc.vector.tensor_add(z3, z2, bft_t)

            # ---- temperature softmax over t (1/TEMP folded into gmt/bft) ----
            mx = small.tile([P, 1], F32)
            nc.vector.tensor_reduce(out=mx, in_=z3, axis=Ax.X, op=Alu.max)
            negmx = small.tile([P, 1], F32)
            nc.vector.tensor_scalar_mul(negmx, mx, -1.0)
            e = small.tile([P, T], F32)
            den = small.tile([P, 1], F32)
            nc.scalar.activation(out=e, in_=z3, func=Act.Exp, bias=negmx,
                                 scale=1.0, accum_out=den)

            # ---- per-patch scalar: gates*spsum+biases (computed early) ----
            rden = small.tile([P, 1], F32)
            nc.vector.reciprocal(out=rden, in_=den)
            fac = small.tile([P, 1], F32)
            nc.vector.tensor_mul(fac, scal2, rden)
            fin = small.tile([P, T], F32)
            nc.vector.tensor_scalar_mul(fin, e, fac)

            nc.sync.dma_start(out=outd[:, :], in_=fin)
    nc.compile()
    return nc


def _get_nc():
    if "nc" not in _NC_CACHE:
        _NC_CACHE["nc"] = _build_nc()
    return _NC_CACHE["nc"]


def _bf16_split_packed(wt):
    """wt [P, S, T] f32 -> (hi bf16, lo fp8e4m3 scaled by 2**LOSH) in packed
    layout [NG, P(partition=s%128), GP*2*T], using uint bit tricks for the
    bf16 rounding (ml_dtypes astype is far too slow for 256MB)."""
    def to_bf16_bits(x):
        u = x.view(np.uint32)
        rounded = u + 0x7FFF + ((u >> 16) & 1)     # round-to-nearest-even
        return (rounded >> 16).astype(np.uint16)

    def to_e4m3(x):
        # fast fp8e4m3 RNE for |x| < 448, with subnormals (ml_dtypes astype
        # is far too slow for 128MB)
        u = x.view(np.uint32)
        s = ((u >> 24) & 0x80).astype(np.uint32)
        mag = u & 0x7FFFFFFF
        r = mag + 0x7FFFF + ((mag >> 20) & 1)
        exp = (r >> 23).astype(np.int32) - 120      # e4m3-biased exponent
        man = (r >> 20) & 0x7
        # subnormal path: round(|x| * 2^9) gives the denormal bits directly
        # (a value of 8 carries into the first normal encoding)
        man_d = np.rint(np.abs(x) * 512.0).astype(np.uint32)
        out = np.where(exp >= 1, (exp.astype(np.uint32) << 3) | man, man_d)
        return (s | out).astype(np.uint8)

    hi_bits = to_bf16_bits(wt)
    hi_f32 = (hi_bits.astype(np.uint32) << 16).view(np.float32)
    lo_fp8 = to_e4m3((wt - hi_f32) * float(2 ** LOSH)).view(ml_dtypes.float8_e4m3)

    def pack(bits):
        # [n, s, t] -> [p, (n, sh, t)] with s = sh*128 + p
        v = bits.reshape(P, 2, P, T).transpose(2, 0, 1, 3)
        return np.ascontiguousarray(v.reshape(P, P * 2 * T))

    return pack(hi_bits).view(NP_BF16), pack(lo_fp8)


def _row_masks():
    if "msk" not in _NC_CACHE:
        m = np.zeros((P, P // 2 + 1, 2), dtype=np.uint8)
        for q in range(P // 2):
            m[2 * q, q, 0] = 1
            m[2 * q + 1, q, 1] = 1
        m[0::2, P // 2, 0] = 1     # even rows
        m[1::2, P // 2, 1] = 1     # odd rows
        _NC_CACHE["msk"] = m
    return _NC_CACHE["msk"]


def _make_in_maps(source_spikes, W_dyn, ln_gamma, ln_beta, gates, biases):
    source_spikes = np.asarray(source_spikes, dtype=np.float32)
    W_dyn = np.asarray(W_dyn, dtype=np.float32)
    ln_gamma = np.asarray(ln_gamma, dtype=np.float32)
    ln_beta = np.asarray(ln_beta, dtype=np.float32)
    gates = np.asarray(gates, dtype=np.float32)
    biases = np.asarray(biases, dtype=np.float32)

    # unfold (matches reference._unfold with kernel=stride=16)
    sp_unf = (
        source_spikes.reshape(B, PH, PATCH, PH, PATCH)
        .transpose(0, 1, 3, 2, 4)
        .reshape(B, N, S)
    )
    sp_unf = np.ascontiguousarray(sp_unf)

    in_maps = []
    for c in range(NCORES):
        b, h = divmod(c, NCORES // B)
        n0 = h * P
        # W^T per patch, split hi/lo bf16, packed to the DMA-friendly layout
        wt = np.ascontiguousarray(W_dyn[b, n0 : n0 + P].transpose(0, 2, 1))
        whi, wlo = _bf16_split_packed(wt)
        spv = np.ascontiguousarray(sp_unf[b, n0 : n0 + P])
        prm = np.empty((P, 2 * T + 2), dtype=np.float32)
        prm[:, 0:T] = ln_gamma / TEMP
        prm[:, T : 2 * T] = ln_beta / TEMP
        prm[:, 2 * T] = gates[n0 : n0 + P]
        prm[:, 2 * T + 1] = biases[n0 : n0 + P]
        spt_np = np.ascontiguousarray(spv.T.astype(NP_BF16))
        in_maps.append({
            "whi": whi,
            "wlo": wlo,
            "spt": spt_np,
            "sptl": np.ascontiguousarray(
                (spv.T * float(2 ** -LOSH)).astype(NP_BF16)),
            "sp": spv,
            "prm": prm,
            "msk": _row_masks(),
        })
    return in_maps


def _assemble(results):
    out_bnt = np.empty((B, N, T), dtype=np.float32)
    for c in range(NCORES):
        b, h = divmod(c, NCORES // B)
        n0 = h * P
        out_bnt[b, n0 : n0 + P] = results[c]["out"]
    # fold (matches reference._fold)
    return np.ascontiguousarray(
        out_bnt.reshape(B, PH, PH, PATCH, PATCH)
        .transpose(0, 1, 3, 2, 4)
        .reshape(B, GRID, GRID)
    )


def run_sharded(inputs: dict, trace: bool = False):
    """Run the SPMD bass kernel on 8 cores. Returns (output, BassKernelResults)."""
    in_maps = _make_in_maps(**inputs)
    nc = _get_nc()
    res = bass_utils.run_bass_kernel_spmd(nc, in_maps, list(range(NCORES)),
                                          trace=trace)
    return _assemble(res.results), res


def kernel(**inputs) -> np.ndarray:
    out, _ = run_sharded(inputs, trace=False)
    return out



# revision 8
# speedup vs baseline: 3.1808x; 3.1808x over previous
"""Trainium2 Bass kernel for nn_AxonalConnections.

Computes, per (batch b, patch n):
    out[t]  = sum_s sp[b,n,s] * W_dyn[b,n,t,s]          (batched matvec, distinct weights)
    out_n   = LayerNorm_T(out) * gamma + beta
    w       = softmax(out_n / TEMP)
    final   = w * (gates[n] * sum_s sp[b,n,s] + biases[n])
    fold -> [B, 256, 256]

Strategy: 8-way shard over (batch b, patch-half); each core owns 128 patches.

Key observation: source_spikes is binary with ~10% density, so the matvec is
a sum of the ~26 active s-columns of W per patch.  The host gathers exactly
those rows (W_dyn[b,n,:,s] for active s), packs them densely per 32-patch
window, and the device does the per-patch segmented sum on the TensorEngine:
each 128-row tile of packed rows is contracted against a [128 rows x 32
patches] 0/1 "staircase" membership mask (lhsT), accumulating into a
[32, 256] PSUM slice per window.  HBM traffic drops from 25.7 MB (dense
bf16+fp8 W stream) to ~2.3 MB (fp16 gathered rows + masks) per core.

Rows ship as fp16 (rel err ~1.6e-3 end to end, measured).  The LayerNorm +
temperature-softmax epilogue is fused: when gamma/beta are constant vectors
(they are: ones/zeros), logits - max(logits) == (x - max(x)) * s with
s = gamma0/(TEMP*std), so one ACT Exp op with per-partition scale/bias does
normalize+softmax straight out of PSUM.  rstd uses exp(-0.5*ln(var+eps)) so
only one ACT table set (natural_log_exp_and_others) is ever loaded.
Unfold/fold, the gather, and shard assembly are host-side numpy.
"""

import os
import sys

for _p in ("/opt/trn_rl_repo",):
    if _p not in sys.path:
        sys.path.insert(0, _p)

import numpy as np

import concourse.bass as bass
import concourse.bacc as bacc
import concourse.tile as tile
from concourse import mybir
from concourse import bass_utils

# Problem constants (hardcoded per contract)
B = 4
GRID = 256
PATCH = 16
PH = GRID // PATCH          # 16 patches per side
N = PH * PH                 # 256 patches
S = PATCH * PATCH           # 256 source pixels per patch
T = 256                     # 256 target pixels per patch
TEMP = 0.1
LN_EPS = 1e-5

NCORES = 8
P = 128                     # patches per core (= SBUF partitions)
NW = 4                      # windows of 32 patches (PSUM col-tiling granularity)
WPATCH = P // NW            # 32 patches per window

F32 = mybir.dt.float32
F16 = mybir.dt.float16
NWARM = int(os.environ.get("BASS_NWARM", "10"))

_NC_CACHE = {}


def _wr_chunks(tiles):
    """Split each window's row-tiles into DMA chunks: small first chunk for a
    fast pipeline ramp, split last window so its tail DMA is short."""
    chunks = []
    for w, tw in enumerate(tiles):
        if w == 0 and tw > 4:
            chunks.append([3, tw - 3])
        elif w == len(tiles) - 1 and tw > 4:
            chunks.append([(tw + 1) // 2, tw // 2])
        else:
            chunks.append([tw])
    return chunks


def _build_nc(tiles, fast, ln_c):
    """tiles: per-window row-tile counts (same across cores); fast: constant
    gamma/beta epilogue; ln_c: log(gamma0/TEMP) for the fused scale."""
    nc = bacc.Bacc("TRN2")
    G = sum(tiles)
    chunks = _wr_chunks(tiles)

    # packed gathered W rows: row r of tile g lives at [r%128, g*256 : +256]
    wr = nc.dram_tensor("wr", [P, G * T], F16, kind="ExternalInput")
    # staircase membership masks, one [128, 32] slab per row-tile
    mk = nc.dram_tensor("mk", [P, G * WPATCH], F16, kind="ExternalInput")
    spv = nc.dram_tensor("spv", [P, S], F32, kind="ExternalInput")
    # params: [gate | bias] fast, + [gamma/TEMP (T) | beta/TEMP (T)] general
    prm_w = 2 if fast else 2 * T + 2
    prm = nc.dram_tensor("prm", [P, prm_w], F32, kind="ExternalInput")
    outd = nc.dram_tensor("out", [P, T], F32, kind="ExternalOutput")

    Alu = mybir.AluOpType
    Act = mybir.ActivationFunctionType
    Ax = mybir.AxisListType

    with tile.TileContext(nc) as tc:
        with (
            tc.tile_pool(name="data", bufs=1) as data,
            tc.tile_pool(name="pspool", bufs=1, space="PSUM") as pspool,
            tc.tile_pool(name="small", bufs=2) as small,
        ):
            # ---- DMA issue (front of both HWDGE rings) ----
            # sync ring: row chunks of even windows + last-window head
            wrt = {}          # (w, c) -> tile
            gg0 = {}          # (w, c) -> first global tile index
            g_run = 0
            order_sync, order_scalar = [], []
            for w, tw in enumerate(tiles):
                c0 = g_run
                for c, cn in enumerate(chunks[w]):
                    wrt[(w, c)] = data.tile([P, cn, T], F16,
                                            name=f"wr{w}_{c}", tag=f"wr{w}_{c}")
                    gg0[(w, c)] = g_run
                    (order_sync if w % 2 == 0 else order_scalar).append((w, c))
                    g_run += cn
                del c0
            # masks: first window's slab first (unblocks first matmul fast)
            t0 = tiles[0]
            mka = data.tile([P, t0, WPATCH], F16, tag="mka")
            mkb = None
            if G > t0:
                mkb = data.tile([P, G - t0, WPATCH], F16, tag="mkb")

            nc.scalar.dma_start(
                out=mka.rearrange("p a b -> p (a b)"),
                in_=mk[:, 0 : t0 * WPATCH])
            for w, c in order_sync:
                g0, cn = gg0[(w, c)], chunks[w][c]
                nc.sync.dma_start(
                    out=wrt[(w, c)].rearrange("p a b -> p (a b)"),
                    in_=wr[:, g0 * T : (g0 + cn) * T])
            if mkb is not None:
                nc.scalar.dma_start(
                    out=mkb.rearrange("p a b -> p (a b)"),
                    in_=mk[:, t0 * WPATCH : G * WPATCH])
            for w, c in order_scalar:
                g0, cn = gg0[(w, c)], chunks[w][c]
                nc.scalar.dma_start(
                    out=wrt[(w, c)].rearrange("p a b -> p (a b)"),
                    in_=wr[:, g0 * T : (g0 + cn) * T])
            spv_t = data.tile([P, S], F32, tag="spv")
            nc.scalar.dma_start(out=spv_t, in_=spv[:, :])
            prm_t = data.tile([P, prm_w], F32, tag="prm")
            nc.scalar.dma_start(out=prm_t, in_=prm[:, :])
            gat_t = prm_t[:, 0:1]
            bia_t = prm_t[:, 1:2]

            # ---- engine warmups (overlap the DMA ramp) ----
            # k folds gamma0/TEMP into the rstd: exp(-0.5*ln(k*(var+eps)))
            # = (gamma0/TEMP) / sqrt(var+eps)  (k=1 on the general path)
            k_fold = float(np.exp(-2.0 * ln_c))
            eps_t = small.tile([P, 1], F32)
            nc.vector.memset(eps_t, LN_EPS * k_fold)
            # ACT: touch Ln+Exp so the single natural_log_exp table set loads
            # during the DMA ramp, not in the epilogue tail
            warm1 = small.tile([P, 1], F32)
            nc.scalar.activation(out=warm1, in_=eps_t, func=Act.Ln)
            warm2 = small.tile([P, 1], F32)
            nc.scalar.activation(out=warm2, in_=eps_t, func=Act.Exp)
            # PE: dummy matmuls to run the HAM activity window up so the real
            # stream executes at 2.4 GHz instead of the cold 1.2 GHz
            wmt = small.tile([P, T], F16)
            nc.vector.memset(wmt, 0.0)
            wps = pspool.tile([P, T], F32, tag="warm")
            for _ in range(NWARM):
                nc.tensor.matmul(wps[0:WPATCH, :], lhsT=wmt[:, 0:WPATCH],
                                 rhs=wmt, start=True, stop=True)

            # per-patch scalar: gates * sum_s(sp) + biases, early
            spsum = small.tile([P, 1], F32)
            nc.vector.tensor_reduce(out=spsum, in_=spv_t, axis=Ax.X, op=Alu.add)
            scal2 = small.tile([P, 1], F32)
            nc.vector.tensor_scalar(out=scal2, in0=spsum, scalar1=gat_t,
                                    scalar2=bia_t, op0=Alu.mult, op1=Alu.add)

            # ---- main pass: per-window segmented sums on the PE ----
            ps = pspool.tile([P, T], F32, tag="acc")
            for w, tw in enumerate(tiles):
                g_base = sum(tiles[:w])
                g = 0
                for c, cn in enumerate(chunks[w]):
                    for i in range(cn):
                        gg = g_base + g
                        mk_sl = (mka[:, gg, :] if gg < t0
                                 else mkb[:, gg - t0, :])
                        nc.tensor.matmul(
                            ps[w * WPATCH : (w + 1) * WPATCH, :],
                            lhsT=mk_sl,
                            rhs=wrt[(w, c)][:, i, :],
                            start=(g == 0),
                            stop=(g == tw - 1),
                            tile_position=(0, w * WPATCH))
                        g += 1

            # ---- fused LayerNorm + temperature softmax epilogue ----
            stats = small.tile([P, 6], F32)
            nc.vector.bn_stats(out=stats, in_=ps)
            mv = small.tile([P, 2], F32)
            nc.vector.bn_aggr(out=mv, in_=stats)
            lnv = small.tile([P, 1], F32)
            nc.scalar.activation(out=lnv, in_=mv[:, 1:2], func=Act.Ln,
                                 scale=k_fold, bias=eps_t)
            e = small.tile([P, T], F32)
            den = small.tile([P, 1], F32)
            if fast:
                # s = gamma0/(TEMP*std); logits - max(logits) == (x - mx)*s,
                # so one Exp straight from PSUM does normalize+softmax
                sfac = small.tile([P, 1], F32)
                nc.scalar.activation(out=sfac, in_=lnv, func=Act.Exp,
                                     scale=-0.5)
                mx = small.tile([P, 1], F32)
                nc.vector.tensor_reduce(out=mx, in_=ps, axis=Ax.X, op=Alu.max)
                nb = small.tile([P, 1], F32)
                nc.vector.tensor_scalar(out=nb, in0=mx, scalar1=sfac,
                                        scalar2=-1.0, op0=Alu.mult,
                                        op1=Alu.mult)
                nc.scalar.activation(out=e, in_=ps, func=Act.Exp, bias=nb,
                                     scale=sfac, accum_out=den)
            else:
                rstd = small.tile([P, 1], F32)
                nc.scalar.activation(out=rstd, in_=lnv, func=Act.Exp,
                                     scale=-0.5)
                z1 = small.tile([P, T], F32)
                nc.vector.tensor_scalar(out=z1, in0=ps, scalar1=mv[:, 0:1],
                                        scalar2=rstd, op0=Alu.subtract,
                                        op1=Alu.mult)
                z2 = small.tile([P, T], F32)
                nc.vector.tensor_mul(z2, z1, prm_t[:, 2 : 2 + T])
                z3 = small.tile([P, T], F32)
                nc.vector.tensor_add(z3, z2, prm_t[:, 2 + T : 2 + 2 * T])
                mx = small.tile([P, 1], F32)
                nc.vector.tensor_reduce(out=mx, in_=z3, axis=Ax.X, op=Alu.max)
                negmx = small.tile([P, 1], F32)
                nc.vector.tensor_scalar_mul(negmx, mx, -1.0)
                nc.scalar.activation(out=e, in_=z3, func=Act.Exp, bias=negmx,
                                     accum_out=den)

            rden = small.tile([P, 1], F32)
            nc.vector.reciprocal(out=rden, in_=den)
            fac = small.tile([P, 1], F32)
            nc.vector.tensor_mul(fac, scal2, rden)
            fin = small.tile([P, T], F32)
            nc.vector.tensor_scalar_mul(fin, e, fac)

            nc.sync.dma_start(out=outd[:, :], in_=fin)
    nc.compile()
    return nc


def _get_nc(tiles, fast, ln_c):
    key = (tuple(tiles), fast, round(float(ln_c), 9))
    if key not in _NC_CACHE:
        _NC_CACHE[key] = _build_nc(list(tiles), fast, ln_c)
    return _NC_CACHE[key]


def _make_in_maps(source_spikes, W_dyn, ln_gamma, ln_beta, gates, biases):
    source_spikes = np.asarray(source_spikes, dtype=np.float32)
    W_dyn = np.asarray(W_dyn, dtype=np.float32)
    ln_gamma = np.asarray(ln_gamma, dtype=np.float32)
    ln_beta = np.asarray(ln_beta, dtype=np.float32)
    gates = np.asarray(gates, dtype=np.float32)
    biases = np.asarray(biases, dtype=np.float32)

    # unfold (matches reference._unfold with kernel=stride=16)
    sp_unf = (
        source_spikes.reshape(B, PH, PATCH, PH, PATCH)
        .transpose(0, 1, 3, 2, 4)
        .reshape(B, N, S)
    )
    sp_unf = np.ascontiguousarray(sp_unf)
    binary = bool(np.all((sp_unf == 0.0) | (sp_unf == 1.0)))

    # per-(core, window) active-row counts -> global per-window tile counts
    active = sp_unf != 0.0
    counts = active.sum(axis=2)                       # [B, N]
    rows_w = counts.reshape(B, 2, NW, WPATCH).sum(axis=3)   # [B, half, NW]
    tiles = [max(1, int(np.ceil(rows_w[:, :, w].max() / P)))
             for w in range(NW)]
    G = sum(tiles)

    fast = bool(
        np.all(ln_gamma == ln_gamma[0]) and np.all(ln_beta == ln_beta[0])
        and ln_gamma[0] > 0.0
    )
    ln_c = float(np.log(ln_gamma[0] / TEMP)) if fast else 0.0

    in_maps = []
    for c in range(NCORES):
        b, h = divmod(c, NCORES // B)
        n0 = h * P
        wrows = np.zeros((P, G * T), dtype=np.float16)
        masks = np.zeros((P, G * WPATCH), dtype=np.float16)
        g_base = 0
        for w in range(NW):
            tw = tiles[w]
            rw = tw * P
            rows = np.zeros((rw, T), dtype=np.float16)
            mrows = np.zeros((rw, WPATCH), dtype=np.float16)
            r = 0
            for j in range(WPATCH):
                n = n0 + w * WPATCH + j
                idx = np.nonzero(active[b, n])[0]
                k = idx.size
                if k:
                    blk = W_dyn[b, n][:, idx].T     # [k, T]
                    if not binary:
                        blk = blk * sp_unf[b, n, idx][:, None]
                    rows[r : r + k] = blk.astype(np.float16)
                    mrows[r : r + k, j] = 1.0
                    r += k
            # [tw*128, T] -> [128, tw, T] partition-major packing
            wrows[:, g_base * T : (g_base + tw) * T] = (
                rows.reshape(tw, P, T).transpose(1, 0, 2).reshape(P, tw * T))
            masks[:, g_base * WPATCH : (g_base + tw) * WPATCH] = (
                mrows.reshape(tw, P, WPATCH).transpose(1, 0, 2)
                .reshape(P, tw * WPATCH))
            g_base += tw

        prm_w = 2 if fast else 2 * T + 2
        prm = np.empty((P, prm_w), dtype=np.float32)
        prm[:, 0] = gates[n0 : n0 + P]
        prm[:, 1] = biases[n0 : n0 + P]
        if not fast:
            prm[:, 2 : 2 + T] = ln_gamma / TEMP
            prm[:, 2 + T : 2 + 2 * T] = ln_beta / TEMP
        in_maps.append({
            "wr": wrows,
            "mk": masks,
            "spv": np.ascontiguousarray(sp_unf[b, n0 : n0 + P]),
            "prm": prm,
        })
    return in_maps, tiles, fast, ln_c


def _assemble(results):
    out_bnt = np.empty((B, N, T), dtype=np.float32)
    for c in range(NCORES):
        b, h = divmod(c, NCORES // B)
        n0 = h * P
        out_bnt[b, n0 : n0 + P] = results[c]["out"]
    # fold (matches reference._fold)
    return np.ascontiguousarray(
        out_bnt.reshape(B, PH, PH, PATCH, PATCH)
        .transpose(0, 1, 3, 2, 4)
        .reshape(B, GRID, GRID)
    )


def run_sharded(inputs: dict, trace: bool = False):
    """Run the SPMD bass kernel on 8 cores. Returns (output, BassKernelResults)."""
    in_maps, tiles, fast, ln_c = _make_in_maps(**inputs)
    nc = _get_nc(tiles, fast, ln_c)
    res = bass_utils.run_bass_kernel_spmd(nc, in_maps, list(range(NCORES)),
                                          trace=trace)
    return _assemble(res.results), res


def kernel(**inputs) -> np.ndarray:
    out, _ = run_sharded(inputs, trace=False)
    return out


# revision 15
# speedup vs baseline: 3.3545x; 1.0546x over previous
"""Trainium2 Bass kernel for nn_AxonalConnections.

Computes, per (batch b, patch n):
    out[t]  = sum_s sp[b,n,s] * W_dyn[b,n,t,s]          (batched matvec, distinct weights)
    out_n   = LayerNorm_T(out) * gamma + beta
    w       = softmax(out_n / TEMP)
    final   = w * (gates[n] * sum_s sp[b,n,s] + biases[n])
    fold -> [B, 256, 256]

Strategy: 8-way shard over (batch b, patch-half); each core owns 128 patches.

Key observation: source_spikes is binary with ~10% density, so the matvec is
a sum of the ~26 active s-columns of W per patch.  The host gathers exactly
those rows (W_dyn[b,n,:,s] for active s), packs them densely per 32-patch
window, and the device does the per-patch segmented sum on the TensorEngine:
each 128-row tile of packed rows is contracted against a [128 rows x 32
patches] 0/1 "staircase" membership mask (lhsT), accumulating into a
[32, 256] PSUM slice per window.  HBM traffic drops from 25.7 MB (dense
bf16+fp8 W stream) to ~2.3 MB (fp16 gathered rows + masks) per core.

Rows ship as fp16 (rel err ~1.6e-3 end to end, measured).  The LayerNorm +
temperature-softmax epilogue is fused: when gamma/beta are constant vectors
(they are: ones/zeros), logits - max(logits) == (x - max(x)) * s with
s = gamma0/(TEMP*std), so one ACT Exp op with per-partition scale/bias does
normalize+softmax straight out of PSUM.  rstd uses exp(-0.5*ln(var+eps)) so
only one ACT table set (natural_log_exp_and_others) is ever loaded.
Unfold/fold, the gather, and shard assembly are host-side numpy.
"""

import os
import sys

for _p in ("/opt/trn_rl_repo",):
    if _p not in sys.path:
        sys.path.insert(0, _p)

import numpy as np

import concourse.bass as bass
import concourse.bacc as bacc
import concourse.tile as tile
from concourse import mybir
from concourse import bass_utils

# Problem constants (hardcoded per contract)
B = 4
GRID = 256
PATCH = 16
PH = GRID // PATCH          # 16 patches per side
N = PH * PH                 # 256 patches
S = PATCH * PATCH           # 256 source pixels per patch
T = 256                     # 256 target pixels per patch
TEMP = 0.1
LN_EPS = 1e-5

NCORES = 8
P = 128                     # patches per core (= SBUF partitions)
NW = 4                      # windows of 32 patches (PSUM col-tiling granularity)
WPATCH = P // NW            # 32 patches per window

F32 = mybir.dt.float32
F16 = mybir.dt.float16
NWARM = int(os.environ.get("BASS_NWARM", "5"))

_NC_CACHE = {}


class _BaccOneActSet(bacc.Bacc):
    """Bacc whose act-table pass is forced to satisfy Ln AND Exp from the
    combined natural_log_exp_and_others set.  The default pass maps Ln ->
    natural_log and Exp -> exp_and_others, so a kernel alternating Ln/Exp
    reloads the ACT tables (~1.3us each) on every switch — 5 loads here,
    several on the critical path."""

    def insert_act_table_loads(self):
        from concourse.hw_specs import get_activation_tables
        from concourse.bacc import _bass_rust
        has_activation = any(
            isinstance(i, mybir.InstActivation)
            for b in self.main_func.blocks
            for i in b.instructions
        )
        if not has_activation:
            return
        both = {mybir.ActivationFunctionType.Ln,
                mybir.ActivationFunctionType.Exp}
        tables = []
        for name, funcs in get_activation_tables(self.m.arch).items():
            if name != "natural_log_exp_and_others":
                funcs = funcs - both
            tables.append((name, funcs))
        _bass_rust.insert_act_table_loads(self, tables)


def _wr_chunks(tiles):
    """Split each window's row-tiles into DMA chunks: small first chunk for a
    fast pipeline ramp, split last window so its tail DMA is short."""
    chunks = []
    for w, tw in enumerate(tiles):
        if w == 0 and tw > 4:
            chunks.append([3, tw - 3])
        elif w == len(tiles) - 1 and tw > 4:
            chunks.append([(tw + 1) // 2, tw // 2])
        else:
            chunks.append([tw])
    return chunks


def _build_nc(tiles, fast, ln_c):
    """tiles: per-window row-tile counts (same across cores); fast: constant
    gamma/beta epilogue; ln_c: log(gamma0/TEMP) for the fused scale."""
    nc = _BaccOneActSet("TRN2")
    G = sum(tiles)
    chunks = _wr_chunks(tiles)

    # packed gathered W rows: row r of tile g lives at [r%128, g*256 : +256]
    wr = nc.dram_tensor("wr", [P, G * T], F16, kind="ExternalInput")
    # staircase membership masks, one [128, 32] slab per row-tile
    mk = nc.dram_tensor("mk", [P, G * WPATCH], F16, kind="ExternalInput")
    # aux: [sp (S) | gate | bias], + [gamma/TEMP (T) | beta/TEMP (T)] general
    aux_w = (S + 2) if fast else (S + 2 + 2 * T)
    aux = nc.dram_tensor("aux", [P, aux_w], F32, kind="ExternalInput")
    outd = nc.dram_tensor("out", [P, T], F32, kind="ExternalOutput")

    Alu = mybir.AluOpType
    Act = mybir.ActivationFunctionType
    Ax = mybir.AxisListType

    with tile.TileContext(nc) as tc:
        with (
            tc.tile_pool(name="data", bufs=1) as data,
            tc.tile_pool(name="pspool", bufs=1, space="PSUM") as pspool,
            tc.tile_pool(name="small", bufs=2) as small,
        ):
            # ---- DMA issue (HWDGE rings for the bulk rows, SWDGE for aux) ----
            wrt = {}          # (w, c) -> tile
            gg0 = {}          # (w, c) -> first global tile index
            g_run = 0
            order_sync, order_scalar = [], []
            for w, tw in enumerate(tiles):
                for c, cn in enumerate(chunks[w]):
                    wrt[(w, c)] = data.tile([P, cn, T], F16,
                                            name=f"wr{w}_{c}", tag=f"wr{w}_{c}")
                    gg0[(w, c)] = g_run
                    (order_sync if w % 2 == 0 else order_scalar).append((w, c))
                    g_run += cn
            mkt = data.tile([P, G, WPATCH], F16, tag="mkt")
            nc.scalar.dma_start(
                out=mkt.rearrange("p a b -> p (a b)"),
                in_=mk[:, :])
            for w, c in order_sync:
                g0, cn = gg0[(w, c)], chunks[w][c]
                nc.sync.dma_start(
                    out=wrt[(w, c)].rearrange("p a b -> p (a b)"),
                    in_=wr[:, g0 * T : (g0 + cn) * T])
            for w, c in order_scalar:
                g0, cn = gg0[(w, c)], chunks[w][c]
                nc.scalar.dma_start(
                    out=wrt[(w, c)].rearrange("p a b -> p (a b)"),
                    in_=wr[:, g0 * T : (g0 + cn) * T])
            # aux is small and off the critical path: issue from the idle
            # GpSimd (SWDGE) so it never contends with the HWDGE rings
            aux_t = data.tile([P, aux_w], F32, tag="aux")
            nc.gpsimd.dma_start(out=aux_t, in_=aux[:, :])
            spv_t = aux_t[:, 0:S]
            gat_t = aux_t[:, S : S + 1]
            bia_t = aux_t[:, S + 1 : S + 2]

            # ---- engine warmups (overlap the DMA ramp) ----
            # k folds gamma0/TEMP into the rstd: exp(-0.5*ln(k*(var+eps)))
            # = (gamma0/TEMP) / sqrt(var+eps)  (k=1 on the general path)
            k_fold = float(np.exp(-2.0 * ln_c))
            eps_t = small.tile([P, 1], F32)
            nc.vector.memset(eps_t, LN_EPS * k_fold)
            # ACT: touch Ln+Exp so the single natural_log_exp table set loads
            # during the DMA ramp, not in the epilogue tail
            warm1 = small.tile([P, 1], F32)
            nc.scalar.activation(out=warm1, in_=eps_t, func=Act.Ln)
            warm2 = small.tile([P, 1], F32)
            nc.scalar.activation(out=warm2, in_=eps_t, func=Act.Exp)
            # PE: dummy matmuls to run the HAM activity window up so the real
            # stream executes at 2.4 GHz instead of the cold 1.2 GHz
            wmt = small.tile([P, 2 * T], F16)
            nc.vector.memset(wmt, 0.0)
            wps = pspool.tile([P, 2 * T], F32, tag="warm")
            for _ in range(NWARM):
                nc.tensor.matmul(wps[0:WPATCH, :], lhsT=wmt[:, 0:WPATCH],
                                 rhs=wmt, start=True, stop=True)

            # per-patch scalar: gates * sum_s(sp) + biases, early
            spsum = small.tile([P, 1], F32)
            nc.vector.tensor_reduce(out=spsum, in_=spv_t, axis=Ax.X, op=Alu.add)
            scal2 = small.tile([P, 1], F32)
            nc.vector.tensor_scalar(out=scal2, in0=spsum, scalar1=gat_t,
                                    scalar2=bia_t, op0=Alu.mult, op1=Alu.add)

            # ---- main pass: per-window segmented sums on the PE ----
            ps = pspool.tile([P, T], F32, tag="acc")
            for w, tw in enumerate(tiles):
                g_base = sum(tiles[:w])
                g = 0
                for c, cn in enumerate(chunks[w]):
                    for i in range(cn):
                        gg = g_base + g
                        mk_sl = mkt[:, gg, :]
                        nc.tensor.matmul(
                            ps[w * WPATCH : (w + 1) * WPATCH, :],
                            lhsT=mk_sl,
                            rhs=wrt[(w, c)][:, i, :],
                            start=(g == 0),
                            stop=(g == tw - 1),
                            tile_position=(0, w * WPATCH))
                        g += 1

            # ---- fused LayerNorm + temperature softmax epilogue ----
            stats = small.tile([P, 6], F32)
            nc.vector.bn_stats(out=stats, in_=ps)
            mv = small.tile([P, 2], F32)
            nc.vector.bn_aggr(out=mv, in_=stats)
            lnv = small.tile([P, 1], F32)
            nc.scalar.activation(out=lnv, in_=mv[:, 1:2], func=Act.Ln,
                                 scale=k_fold, bias=eps_t)
            e = small.tile([P, T], F32)
            den = small.tile([P, 1], F32)
            if fast:
                # s = gamma0/(TEMP*std); logits - max(logits) == (x - mx)*s,
                # so one Exp straight from PSUM does normalize+softmax
                sfac = small.tile([P, 1], F32)
                nc.scalar.activation(out=sfac, in_=lnv, func=Act.Exp,
                                     scale=-0.5)
                mx = small.tile([P, 1], F32)
                nc.vector.tensor_reduce(out=mx, in_=ps, axis=Ax.X, op=Alu.max)
                nb = small.tile([P, 1], F32)
                nc.vector.tensor_scalar(out=nb, in0=mx, scalar1=sfac,
                                        scalar2=-1.0, op0=Alu.mult,
                                        op1=Alu.mult)
                nc.scalar.activation(out=e, in_=ps, func=Act.Exp, bias=nb,
                                     scale=sfac, accum_out=den)
            else:
                rstd = small.tile([P, 1], F32)
                nc.scalar.activation(out=rstd, in_=lnv, func=Act.Exp,
                                     scale=-0.5)
                z1 = small.tile([P, T], F32)
                nc.vector.tensor_scalar(out=z1, in0=ps, scalar1=mv[:, 0:1],
                                        scalar2=rstd, op0=Alu.subtract,
                                        op1=Alu.mult)
                z2 = small.tile([P, T], F32)
                nc.vector.tensor_mul(z2, z1, aux_t[:, S + 2 : S + 2 + T])
                z3 = small.tile([P, T], F32)
                nc.vector.tensor_add(z3, z2, aux_t[:, S + 2 + T : S + 2 + 2 * T])
                mx = small.tile([P, 1], F32)
                nc.vector.tensor_reduce(out=mx, in_=z3, axis=Ax.X, op=Alu.max)
                negmx = small.tile([P, 1], F32)
                nc.vector.tensor_scalar_mul(negmx, mx, -1.0)
                nc.scalar.activation(out=e, in_=z3, func=Act.Exp, bias=negmx,
                                     accum_out=den)

            rden = small.tile([P, 1], F32)
            nc.vector.reciprocal(out=rden, in_=den)
            fac = small.tile([P, 1], F32)
            nc.vector.tensor_mul(fac, scal2, rden)
            fin = small.tile([P, T], F32)
            nc.vector.tensor_scalar_mul(fin, e, fac)

            nc.sync.dma_start(out=outd[:, :], in_=fin)
    nc.compile()
    return nc


def _get_nc(tiles, fast, ln_c):
    key = (tuple(tiles), fast, round(float(ln_c), 9))
    if key not in _NC_CACHE:
        _NC_CACHE[key] = _build_nc(list(tiles), fast, ln_c)
    return _NC_CACHE[key]


def _make_in_maps(source_spikes, W_dyn, ln_gamma, ln_beta, gates, biases):
    source_spikes = np.asarray(source_spikes, dtype=np.float32)
    W_dyn = np.asarray(W_dyn, dtype=np.float32)
    ln_gamma = np.asarray(ln_gamma, dtype=np.float32)
    ln_beta = np.asarray(ln_beta, dtype=np.float32)
    gates = np.asarray(gates, dtype=np.float32)
    biases = np.asarray(biases, dtype=np.float32)

    # unfold (matches reference._unfold with kernel=stride=16)
    sp_unf = (
        source_spikes.reshape(B, PH, PATCH, PH, PATCH)
        .transpose(0, 1, 3, 2, 4)
        .reshape(B, N, S)
    )
    sp_unf = np.ascontiguousarray(sp_unf)
    binary = bool(np.all((sp_unf == 0.0) | (sp_unf == 1.0)))

    # per-(core, window) active-row counts -> global per-window tile counts
    active = sp_unf != 0.0
    counts = active.sum(axis=2)                       # [B, N]
    rows_w = counts.reshape(B, 2, NW, WPATCH).sum(axis=3)   # [B, half, NW]
    tiles = [max(1, int(np.ceil(rows_w[:, :, w].max() / P)))
             for w in range(NW)]
    G = sum(tiles)

    fast = bool(
        np.all(ln_gamma == ln_gamma[0]) and np.all(ln_beta == ln_beta[0])
        and ln_gamma[0] > 0.0
    )
    ln_c = float(np.log(ln_gamma[0] / TEMP)) if fast else 0.0

    in_maps = []
    for c in range(NCORES):
        b, h = divmod(c, NCORES // B)
        n0 = h * P
        wrows = np.zeros((P, G * T), dtype=np.float16)
        masks = np.zeros((P, G * WPATCH), dtype=np.float16)
        g_base = 0
        for w in range(NW):
            tw = tiles[w]
            rw = tw * P
            rows = np.zeros((rw, T), dtype=np.float16)
            mrows = np.zeros((rw, WPATCH), dtype=np.float16)
            r = 0
            for j in range(WPATCH):
                n = n0 + w * WPATCH + j
                idx = np.nonzero(active[b, n])[0]
                k = idx.size
                if k:
                    blk = W_dyn[b, n][:, idx].T     # [k, T]
                    if not binary:
                        blk = blk * sp_unf[b, n, idx][:, None]
                    rows[r : r + k] = blk.astype(np.float16)
                    mrows[r : r + k, j] = 1.0
                    r += k
            # [tw*128, T] -> [128, tw, T] partition-major packing
            wrows[:, g_base * T : (g_base + tw) * T] = (
                rows.reshape(tw, P, T).transpose(1, 0, 2).reshape(P, tw * T))
            masks[:, g_base * WPATCH : (g_base + tw) * WPATCH] = (
                mrows.reshape(tw, P, WPATCH).transpose(1, 0, 2)
                .reshape(P, tw * WPATCH))
            g_base += tw

        aux_w = (S + 2) if fast else (S + 2 + 2 * T)
        aux = np.empty((P, aux_w), dtype=np.float32)
        aux[:, 0:S] = sp_unf[b, n0 : n0 + P]
        aux[:, S] = gates[n0 : n0 + P]
        aux[:, S + 1] = biases[n0 : n0 + P]
        if not fast:
            aux[:, S + 2 : S + 2 + T] = ln_gamma / TEMP
            aux[:, S + 2 + T :] = ln_beta / TEMP
        in_maps.append({
            "wr": wrows,
            "mk": masks,
            "aux": aux,
        })
    return in_maps, tiles, fast, ln_c


def _assemble(results):
    out_bnt = np.empty((B, N, T), dtype=np.float32)
    for c in range(NCORES):
        b, h = divmod(c, NCORES // B)
        n0 = h * P
        out_bnt[b, n0 : n0 + P] = results[c]["out"]
    # fold (matches reference._fold)
    return np.ascontiguousarray(
        out_bnt.reshape(B, PH, PH, PATCH, PATCH)
        .transpose(0, 1, 3, 2, 4)
        .reshape(B, GRID, GRID)
    )


def run_sharded(inputs: dict, trace: bool = False):
    """Run the SPMD bass kernel on 8 cores. Returns (output, BassKernelResults)."""
    in_maps, tiles, fast, ln_c = _make_in_maps(**inputs)
    nc = _get_nc(tiles, fast, ln_c)
    res = bass_utils.run_bass_kernel_spmd(nc, in_maps, list(range(NCORES)),
                                          trace=trace)
    return _assemble(res.results), res


def kernel(**inputs) -> np.ndarray:
    out, _ = run_sharded(inputs, trace=False)
    return out
